# revision 1
# baseline (speedup 1.0000x reference)
"""Trainium2 Bass kernel for nn_CombinatorialClassifier (segment_reduce), v2.

Strategy (8 NeuronCores, tensor-parallel over the num_partitionings axis):
  Core i owns partitionings {2i, 2i+1} (a [2000, 2048] W slice).

  v2 speedup over the baseline: the gpsimd ap_gather moves one 32-bit
  word per index per Q7 lane, so we pack TWO fp16 batch values (an
  even/odd batch pair) into each gathered word, halving the gather
  index stream (the baseline bottleneck). Layout:

  - logits = x @ Wshard.T + b per half (partitioning), softmax'd to
    fp16 probs [64, 2000] (slots [0:1000]=p0, [1000:2000]=p1).
  - A 0/1 "selection" matmul replicates/reorders probs into a packed
    tile [128, 2000 slots x 2 fp16]: row 16q+l holds the batch pair
    (2*bp, 2*bp+1), bp = 16*(q&1)+l; Q7 core q serves class quarter
    q>>1. Even/odd halves land via strided fp16 copies from PSUM.
  - ap_gather (d=2, fp16) picks the packed pair per (class, p); the
    two partitionings are summed on DVE in fp16 (2x mode) and the
    [128, 25000] fp16 partial goes to DRAM.
  - W streams half-by-half so the p0 gather overlaps the p1 matmul.

  Host: reassemble the 8 per-core partials (batch-pair x quarter
  layout), sum over cores, normalize over classes, log.
"""

import os
from contextlib import ExitStack

import numpy as np

import concourse.bacc as bacc
import concourse.mybir as mybir
import concourse.tile as tile
from concourse import bass_utils

B, P, K, C, D = 64, 16, 1000, 50000, 2048
ESP = 1e-20
NCORES = 8
NLOC = 2 * K             # local logits width (2000)
NT = 500                 # matmul N-tile (PSUM bank: 500 fp32 <= 512)
DCH = D // 128           # 16 contraction chunks of 128
CQ4 = C // 4             # classes per Q7-core quarter (12500)
JC = 2048                # classes per gather call

CHUNKS = []
_c = 0
while _c < CQ4:
    CHUNKS.append(min(JC, CQ4 - _c))
    _c += JC
PCHUNKS = [(n + 15) // 16 * 16 for n in CHUNKS]   # padded to idx wrap
SCOLS = [n // 16 for n in PCHUNKS]                # int16 idx cols per call
IDXCOLS = 2 * sum(SCOLS)                          # both partitionings
GTOT = 2 * sum(PCHUNKS)                           # fp16 cols of g0

_F32 = mybir.dt.float32
_F16 = mybir.dt.float16
_I16 = mybir.dt.int16

_CACHE = {}
LAST_RESULTS = None


def _build_nc():
    nc = bacc.Bacc(
        "TRN2",
        target_bir_lowering=False,
        debug=False,
        enable_asserts=False,
        num_devices=NCORES,
    )
    xT_d = nc.dram_tensor("xT", [D, B], _F16, kind="ExternalInput")
    # [2, D+1, K] flattened: per half h, rows h*(D+1)+d = W.T row, last row bias
    wtb_d = nc.dram_tensor("wtb", [2 * (D + 1), K], _F16, kind="ExternalInput")
    sel_d = nc.dram_tensor("sel", [B, 256], _F16, kind="ExternalInput")
    idx_d = nc.dram_tensor("idx", [128, IDXCOLS], _I16, kind="ExternalInput")
    out_d = nc.dram_tensor("part_out", [128, 2 * CQ4], _F16, kind="ExternalOutput")

    with tile.TileContext(nc) as tc, ExitStack() as ctx:
        const = ctx.enter_context(tc.tile_pool(name="const", bufs=1))
        wpool = ctx.enter_context(tc.tile_pool(name="w", bufs=3))
        spool = ctx.enter_context(tc.tile_pool(name="stats", bufs=1))
        gpool = ctx.enter_context(tc.tile_pool(name="g", bufs=2))
        apool = ctx.enter_context(tc.tile_pool(name="a", bufs=2))
        psum = ctx.enter_context(
            tc.tile_pool(name="psum", bufs=1, space="PSUM")
        )
        psum2 = ctx.enter_context(
            tc.tile_pool(name="psum2", bufs=2, space="PSUM")
        )

        xt = const.tile([128, DCH, B], _F16)
        nc.sync.dma_start(xt[:], xT_d.ap().rearrange("(c p) b -> p c b", p=128))
        ones = const.tile([1, B], _F16)
        nc.vector.memset(ones[:], 1.0)
        bias = const.tile([1, NLOC], _F16)
        nc.sync.dma_start(bias[:, 0:K], wtb_d[D : D + 1, :])
        nc.sync.dma_start(bias[:, K:NLOC], wtb_d[2 * D + 1 : 2 * D + 2, :])
        sel_sb = const.tile([B, 256], _F16)
        nc.sync.dma_start(sel_sb[:], sel_d.ap())
        idx_sb = const.tile([128, IDXCOLS], _I16)
        nc.sync.dma_start(idx_sb[:], idx_d.ap())

        probs = const.tile([B, NLOC], _F16)
        packed = const.tile([128, 2 * NLOC], _F16)
        g0 = const.tile([128, GTOT], _F16)

        mx = spool.tile([B, 4], _F32)
        neg = spool.tile([B, 2], _F32)
        sacc = spool.tile([B, 4], _F32)
        rec = spool.tile([B, 2], _F32)

        ps = [
            psum.tile([B, NT], _F32, tag=f"ps{n}", name=f"ps{n}")
            for n in range(4)
        ]

        def half(h):
            # ---- logits half h: x @ Wshard[:, hK:(h+1)K].T + b ----
            with nc.named_scope(f"mm{h}"):
                for j in range(DCH):
                    wt = wpool.tile([128, K], _F16, tag="wt", name="wt")
                    nc.sync.dma_start(
                        wt[:],
                        wtb_d[h * (D + 1) + 128 * j : h * (D + 1) + 128 * (j + 1), :],
                    )
                    for n in range(2):
                        nc.tensor.matmul(
                            ps[2 * h + n][:],
                            xt[:, j, :],
                            wt[:, NT * n : NT * (n + 1)],
                            start=(j == 0),
                            stop=False,
                        )
                for n in range(2):
                    nc.tensor.matmul(
                        ps[2 * h + n][:],
                        ones[:],
                        bias[:, K * h + NT * n : K * h + NT * (n + 1)],
                        start=False,
                        stop=True,
                    )

            # ---- softmax half h -> probs[:, hK:(h+1)K] fp16 ----
            with nc.named_scope(f"sm{h}"):
                for n in range(2):
                    nc.vector.reduce_max(
                        mx[:, 2 * h + n : 2 * h + n + 1],
                        ps[2 * h + n][:],
                        axis=mybir.AxisListType.X,
                    )
                nc.vector.tensor_tensor(
                    neg[:, h : h + 1],
                    mx[:, 2 * h : 2 * h + 1],
                    mx[:, 2 * h + 1 : 2 * h + 2],
                    op=mybir.AluOpType.max,
                )
                nc.vector.tensor_scalar_mul(
                    neg[:, h : h + 1], neg[:, h : h + 1], -1.0
                )
                for n in range(2):
                    nc.scalar.activation(
                        probs[:, K * h + NT * n : K * h + NT * (n + 1)],
                        ps[2 * h + n][:],
                        mybir.ActivationFunctionType.Exp,
                        bias=neg[:, h : h + 1],
                        accum_out=sacc[:, 2 * h + n : 2 * h + n + 1],
                    )
                nc.vector.tensor_tensor(
                    rec[:, h : h + 1],
                    sacc[:, 2 * h : 2 * h + 1],
                    sacc[:, 2 * h + 1 : 2 * h + 2],
                    op=mybir.AluOpType.add,
                )
                nc.vector.reciprocal(rec[:, h : h + 1], rec[:, h : h + 1])
                nc.vector.tensor_scalar_mul(
                    probs[:, K * h : K * (h + 1)],
                    probs[:, K * h : K * (h + 1)],
                    rec[:, h : h + 1],
                )

            # ---- pack half h: packed[r, 2k+j] = probs[2*bp(r)+j, k] ----
            with nc.named_scope(f"pack{h}"):
                for n in range(2 * h, 2 * h + 2):
                    ppe = psum2.tile([128, NT], _F32, tag="ppe", name="ppe")
                    ppo = psum2.tile([128, NT], _F32, tag="ppo", name="ppo")
                    nc.tensor.matmul(
                        ppe[:], sel_sb[:, 0:128],
                        probs[:, NT * n : NT * (n + 1)],
                        start=True, stop=True,
                    )
                    nc.tensor.matmul(
                        ppo[:], sel_sb[:, 128:256],
                        probs[:, NT * n : NT * (n + 1)],
                        start=True, stop=True,
                    )
                    nc.scalar.copy(
                        packed[:, 2 * NT * n : 2 * NT * (n + 1) : 2], ppe[:]
                    )
                    nc.vector.tensor_copy(
                        packed[:, 2 * NT * n + 1 : 2 * NT * (n + 1) : 2], ppo[:]
                    )

        def gather_p(p, and_out):
            src = packed[:, 2 * K * p : 2 * K * (p + 1)]
            goff = 0
            coff = p * sum(SCOLS)
            for ci, (nv, npad) in enumerate(zip(CHUNKS, PCHUNKS)):
                S = SCOLS[ci]
                if not and_out:
                    dst = g0[:, 2 * goff : 2 * goff + 2 * npad]
                else:
                    dst_t = gpool.tile([128, 2 * JC], _F16, tag="g1", name="g1")
                    dst = dst_t[:, 0 : 2 * npad]
                nc.gpsimd.ap_gather(
                    dst,
                    src,
                    idx_sb[:, coff : coff + S],
                    channels=128,
                    num_elems=K,
                    d=2,
                    num_idxs=npad,
                )
                if and_out:
                    a = apool.tile([128, 2 * JC], _F16, tag="a", name="a")
                    nc.vector.tensor_add(
                        a[:, 0 : 2 * npad],
                        g0[:, 2 * goff : 2 * goff + 2 * npad],
                        dst,
                    )
                    nc.sync.dma_start(
                        out_d[:, 2 * goff : 2 * goff + 2 * nv],
                        a[:, 0 : 2 * nv],
                    )
                goff += npad
                coff += S

        half(0)
        with nc.named_scope("gather_p0"):
            gather_p(0, and_out=False)
        half(1)
        with nc.named_scope("gather_p1"):
            gather_p(1, and_out=True)

    nc.compile()
    return nc


def _host_inputs(x, W, b, part):
    """Per-core input maps: xT, wtb (half-major W.T + bias), sel, idx."""
    xT = np.ascontiguousarray(x.T.astype(np.float16))
    part = np.asarray(part).astype(np.int64, copy=False)

    # sel: [64, 256]; col r (even) / 128+r (odd), r = 16q+l -> bp = 16*(q&1)+l
    sel = np.zeros((B, 256), np.float16)
    for q in range(8):
        hh = q & 1
        for l in range(16):
            r = 16 * q + l
            bp = 16 * hh + l
            sel[2 * bp, r] = 1.0
            sel[2 * bp + 1, 128 + r] = 1.0

    in_maps = []
    for i in range(NCORES):
        r0 = NLOC * i
        wtb = np.empty((2 * (D + 1), K), np.float16)
        for h in range(2):
            rows = slice(h * (D + 1), h * (D + 1) + D)
            wtb[rows] = W[r0 + h * K : r0 + (h + 1) * K].T
            wtb[h * (D + 1) + D] = b[r0 + h * K : r0 + (h + 1) * K]

        pab = np.stack(
            [part[2 * i] - (2 * i) * K, part[2 * i + 1] - (2 * i + 1) * K]
        ).astype(np.int16)  # [2, C] in [0, K)

        idxh = np.zeros((128, IDXCOLS), np.int16)
        for p in range(2):
            coff = p * sum(SCOLS)
            for q in range(8):
                jq = q >> 1
                off = 0
                cc = coff
                for ci, (nv, npad) in enumerate(zip(CHUNKS, PCHUNKS)):
                    S = SCOLS[ci]
                    vals = np.zeros(npad, np.int16)
                    vals[:nv] = pab[p, jq * CQ4 + off : jq * CQ4 + off + nv]
                    blk = vals.reshape(S, 16).T
                    idxh[16 * q : 16 * q + 16, cc : cc + S] = blk
                    off += nv
                    cc += S
        in_maps.append({"xT": xT, "wtb": wtb, "sel": sel, "idx": idxh})
    return in_maps


def kernel(**inputs):
    global LAST_RESULTS
    x = np.asarray(inputs["input"], dtype=np.float32)
    W = np.asarray(inputs["W"], dtype=np.float32)
    b = np.asarray(inputs["b"], dtype=np.float32)
    part = np.asarray(inputs["partitionings"])
    assert x.shape == (B, D) and W.shape == (P * K, D)

    if "nc" not in _CACHE:
        _CACHE["nc"] = _build_nc()
    nc = _CACHE["nc"]

    in_maps = _host_inputs(x, W, b, part)
    trace = bool(int(os.environ.get("BASSK_TRACE", "0")))
    res = bass_utils.run_bass_kernel_spmd(
        nc,
        in_maps,
        core_ids=list(range(NCORES)),
        trace=trace,
        tmpdir=os.environ.get("BASSK_TRACE_DIR") or None,
    )
    LAST_RESULTS = res

    acc = np.zeros((B, C), np.float32)
    for i in range(NCORES):
        v = res.results[i]["part_out"].astype(np.float32)
        v = v.reshape(4, 2, 16, CQ4, 2)          # [jq, h, l, c, j]
        acc += v.transpose(1, 2, 4, 0, 3).reshape(B, C)
    tot = acc.sum(axis=1, keepdims=True)
    return np.log(acc / tot + ESP).astype(np.float32)



# revision 6
# speedup vs baseline: 1.8425x; 1.8425x over previous
"""Trainium2 Bass kernel for nn_CombinatorialClassifier (segment_reduce), v4.

Strategy (8 NeuronCores, tensor-parallel over the num_partitionings axis):
  Core i owns partitionings {2i, 2i+1} (a [2000, 2048] W slice).

  The segment-gather (out[b,c] = sum_p probs_p[b, idx_p(c)]) runs as a
  SWDGE dma_gather over host-sorted classes:

  - Host sorts each partitioning's classes by partition id k and pads
    every k-run to a multiple of 8, so each group of 8 consecutive
    sorted positions shares one k ("pure oct").
  - Device computes logits = x @ W.T + b, then exp(x - max + ln 128)
    WITHOUT the 1/sum normalization (the per-batch softmax sums ship to
    the host, which applies them during reassembly — exact in fp32).
  - TensorE transposes the fp8-quantized exp values into probsT
    [128, 8, 64] (col k at partition k&127, rank k>>7); 8 strided
    copies replicate each col into an oct row table [1024 rows, 512B]
    (row k = col_k fp8 x8) which is DMA'd to DRAM.
  - dma_gather (HBM source, non-transpose, elem 512B) moves one oct
    row per descriptor: dst[i%128, i//128, 64*j+b] = batch b of sorted
    position 8i+j. 4 calls per half (per-call descriptor-ring cap),
    round-robin over 4 SWDGE queues (safe: non-transpose gathers don't
    touch the shared XBAR transpose unit, unlike transpose mode which
    corrupts data when two queues run concurrently).
  - Sorted fp8 partials [128, TOT/128*512] go to DRAM; the host
    un-permutes (one fancy-index gather per partitioning), applies
    1/sum, accumulates all 16 in fp32, normalizes and takes the log.
"""

import os
from contextlib import ExitStack

import numpy as np

import concourse.bacc as bacc
import concourse.mybir as mybir
import concourse.tile as tile
from concourse import bass_utils

B, P, K, C, D = 64, 16, 1000, 50000, 2048
ESP = 1e-20
NCORES = 8
DCH = D // 128           # 16 contraction chunks of 128
NT = 500                 # matmul N-tile (PSUM bank: 500 fp32 <= 512)
WQ = 4                   # W j-chunks per DMA tile

TOT = 55296              # padded sorted positions per partitioning (8*6912)
NOCT = TOT // 8          # 6912 oct rows per partitioning
OCT_CH = (1920, 1920, 1920, 1152)   # per-call octs (desc ring cap ~2016)
IDXW = NOCT // 16        # idx columns per partitioning (432)
NQ = 4                   # SWDGE queues
LN_SCALE = 4.852030263919617        # ln(128): exp scale for fp8 range

_F32 = mybir.dt.float32
_F16 = mybir.dt.float16
_F8 = mybir.dt.float8e4
_I16 = mybir.dt.int16

_CACHE = {}
LAST_RESULTS = None


def _build_nc():
    nc = bacc.Bacc(
        "TRN2",
        target_bir_lowering=False,
        debug=False,
        enable_asserts=False,
        num_devices=NCORES,
        num_swdge_queues=NQ,
    )
    xT_d = nc.dram_tensor("xT", [D, B], _F16, kind="ExternalInput")
    # [2, D+1, K] flattened: per half h, rows h*(D+1)+d = W.T row, last row bias
    wtb_d = nc.dram_tensor("wtb", [2 * (D + 1), K], _F16, kind="ExternalInput")
    id64_d = nc.dram_tensor("id64", [B, B], _F16, kind="ExternalInput")
    idx_d = nc.dram_tensor("idx", [128, 2 * IDXW], _I16, kind="ExternalInput")
    tab_d = [
        nc.dram_tensor(f"tab{h}", [1024, 512], _F8, kind="Internal")
        for h in range(2)
    ]
    out_d = [
        nc.dram_tensor(f"g{h}", [128, (NOCT // 128) * 512], _F8,
                       kind="ExternalOutput")
        for h in range(2)
    ]
    sacc_d = nc.dram_tensor("sacc", [B, 4], _F32, kind="ExternalOutput")

    with tile.TileContext(nc) as tc, ExitStack() as ctx:
        const = ctx.enter_context(tc.tile_pool(name="const", bufs=1))
        wpool = ctx.enter_context(tc.tile_pool(name="w", bufs=6))
        spool = ctx.enter_context(tc.tile_pool(name="stats", bufs=1))
        ppool = ctx.enter_context(tc.tile_pool(name="probs", bufs=2))
        tpool = ctx.enter_context(tc.tile_pool(name="pt", bufs=2))
        tabpool = ctx.enter_context(tc.tile_pool(name="tab", bufs=2))
        gpool = ctx.enter_context(tc.tile_pool(name="g", bufs=5))
        psum = ctx.enter_context(tc.tile_pool(name="psum", bufs=1, space="PSUM"))
        psum2 = ctx.enter_context(tc.tile_pool(name="psum2", bufs=2, space="PSUM"))

        xt = const.tile([128, DCH, B], _F16)
        nc.sync.dma_start(xt[:], xT_d.ap().rearrange("(c p) b -> p c b", p=128))
        ones = const.tile([1, B], _F16)
        nc.vector.memset(ones[:], 1.0)
        bias = const.tile([1, 2, K], _F16)
        nc.sync.dma_start(bias[:, 0, :], wtb_d[D : D + 1, :])
        nc.sync.dma_start(bias[:, 1, :], wtb_d[2 * D + 1 : 2 * D + 2, :])
        id64 = const.tile([B, B], _F16)
        nc.sync.dma_start(id64[:], id64_d.ap())
        idx_sb = const.tile([128, 2 * IDXW], _I16)
        nc.sync.dma_start(idx_sb[:], idx_d.ap())

        mx = spool.tile([B, 4], _F32)
        neg = spool.tile([B, 2], _F32)
        sacc = spool.tile([B, 4], _F32)

        ps = [
            psum.tile([B, NT], _F32, tag=f"ps{n}", name=f"ps{n}")
            for n in range(4)
        ]

        def half(h):
            # ---- logits half h: x @ Wshard[:, hK:(h+1)K].T (+ b) ----
            with nc.named_scope(f"mm{h}"):
                for jq in range(DCH // WQ):
                    wt = wpool.tile([128, WQ, K], _F16, tag="wt", name="wt")
                    eng = nc.sync if jq % 2 == 0 else nc.scalar
                    r0 = h * (D + 1) + 128 * WQ * jq
                    eng.dma_start(
                        wt[:],
                        wtb_d[r0 : r0 + 128 * WQ, :].rearrange(
                            "(c p) k -> p c k", p=128
                        ),
                    )
                    for j in range(WQ):
                        for n in range(2):
                            nc.tensor.matmul(
                                ps[2 * h + n][:],
                                xt[:, WQ * jq + j, :],
                                wt[:, j, NT * n : NT * (n + 1)],
                                start=(jq == 0 and j == 0),
                                stop=False,
                            )
                for n in range(2):
                    nc.tensor.matmul(
                        ps[2 * h + n][:],
                        ones[:],
                        bias[:, h, NT * n : NT * (n + 1)],
                        start=False,
                        stop=True,
                    )

            # ---- scaled exp half h -> probs fp16 [64, K] (unnormalized) ----
            probs = ppool.tile([B, K], _F16, tag="probs", name="probs")
            with nc.named_scope(f"sm{h}"):
                for n in range(2):
                    nc.vector.reduce_max(
                        mx[:, 2 * h + n : 2 * h + n + 1],
                        ps[2 * h + n][:],
                        axis=mybir.AxisListType.X,
                    )
                nc.vector.tensor_tensor(
                    neg[:, h : h + 1],
                    mx[:, 2 * h : 2 * h + 1],
                    mx[:, 2 * h + 1 : 2 * h + 2],
                    op=mybir.AluOpType.max,
                )
                # neg = ln(128) - max  (exp scaled into fp8 range)
                nc.vector.tensor_scalar(
                    neg[:, h : h + 1],
                    neg[:, h : h + 1],
                    -1.0,
                    LN_SCALE,
                    op0=mybir.AluOpType.mult,
                    op1=mybir.AluOpType.add,
                )
                for n in range(2):
                    nc.scalar.activation(
                        probs[:, NT * n : NT * (n + 1)],
                        ps[2 * h + n][:],
                        mybir.ActivationFunctionType.Exp,
                        bias=neg[:, h : h + 1],
                        accum_out=sacc[:, 2 * h + n : 2 * h + n + 1],
                    )

            # ---- transpose: probsT[p, r, b] = probs[b, 128r + p] (fp8) ----
            probsT = tpool.tile([128, 8, B], _F8, tag="pT", name="pT")
            with nc.named_scope(f"tr{h}"):
                for r in range(8):
                    w = min(128, K - 128 * r)
                    pt = psum2.tile([128, B], _F32, tag="pt", name="pt")
                    nc.tensor.matmul(
                        pt[0:w, :],
                        probs[:, 128 * r : 128 * r + w],
                        id64[:],
                        start=True,
                        stop=True,
                    )
                    nc.scalar.copy(probsT[0:w, r, :], pt[0:w, :])

            # ---- oct table: tab[p, kc, j, b] = probsT[p, kc, b], to DRAM ----
            tab = tabpool.tile([128, 8, 8, B], _F8, tag="tab", name="tab")
            with nc.named_scope(f"tab{h}"):
                for j in range(8):
                    eng = nc.scalar if j % 2 == 0 else nc.vector
                    if eng is nc.scalar:
                        eng.copy(tab[:, :, j, :], probsT[:, :, :])
                    else:
                        eng.tensor_copy(tab[:, :, j, :], probsT[:, :, :])
                nc.sync.dma_start(
                    tab_d[h].ap().rearrange("(c p) e -> p c e", p=128),
                    tab[:],
                )

            # ---- gathers: 4 calls, round-robin queues ----
            with nc.named_scope(f"gather{h}"):
                c0 = 0
                for ci, n in enumerate(OCT_CH):
                    dst = gpool.tile(
                        [128, n // 128, 512], _F8, tag=f"dst{n}", name="dst"
                    )
                    nc.gpsimd.dma_gather(
                        dst[:],
                        tab_d[h].ap(),
                        idx_sb[:, h * IDXW + c0 // 16 : h * IDXW + (c0 + n) // 16],
                        n,
                        n,
                        512,
                        queue_num=(h * len(OCT_CH) + ci) % NQ,
                    )
                    eng = nc.sync if ci % 2 == 0 else nc.scalar
                    eng.dma_start(
                        out_d[h][:, (c0 // 128) * 512 : ((c0 + n) // 128) * 512],
                        dst[:],
                    )
                    c0 += n

        half(0)
        half(1)
        nc.sync.dma_start(sacc_d.ap(), sacc[:])

    nc.compile()
    return nc


def _oct_prep(kval):
    """Pure-oct sorted layout for one partitioning.

    kval: [C] ints in [0, K). Pads every k-run to a multiple of 8 and the
    total to TOT. Returns (rows int16 [NOCT], posmap int64 [C]).
    """
    counts = np.bincount(kval, minlength=K)
    pad = (8 - counts % 8) % 8
    total = int(counts.sum() + pad.sum())
    assert total <= TOT, f"pathological partition map: {total} > {TOT}"
    order = np.argsort(kval, kind="stable")
    n_pad = counts + pad
    n_pad[K - 1] += TOT - total
    starts = np.concatenate(([0], np.cumsum(n_pad)[:-1]))
    src_starts = np.concatenate(([0], np.cumsum(counts)[:-1]))
    within = np.arange(C) - np.repeat(src_starts, counts)
    pos = np.repeat(starts, counts) + within          # position of order[j]
    rows = np.repeat(np.arange(K), n_pad)[::8]        # k of each oct
    posmap = np.empty(C, np.int64)
    posmap[order] = pos
    return rows.astype(np.int16), posmap


def _host_inputs(x, W, b, part):
    xT = np.ascontiguousarray(x.T.astype(np.float16))
    id64 = np.eye(B, dtype=np.float16)
    part = np.asarray(part).astype(np.int64, copy=False)

    in_maps, posmaps = [], []
    for i in range(NCORES):
        r0 = 2 * K * i
        wtb = np.empty((2 * (D + 1), K), np.float16)
        for h in range(2):
            rows = slice(h * (D + 1), h * (D + 1) + D)
            wtb[rows] = W[r0 + h * K : r0 + (h + 1) * K].T
            wtb[h * (D + 1) + D] = b[r0 + h * K : r0 + (h + 1) * K]

        idxh = np.zeros((128, 2 * IDXW), np.int16)
        pm = []
        for h in range(2):
            kval = (part[2 * i + h] - (2 * i + h) * K).astype(np.int64)
            rows_o, posmap = _oct_prep(kval)
            blk = rows_o.reshape(IDXW, 16).T
            for q in range(8):
                idxh[16 * q : 16 * q + 16, h * IDXW : (h + 1) * IDXW] = blk
            pm.append(posmap)
        posmaps.append(pm)
        in_maps.append({"xT": xT, "wtb": wtb, "id64": id64, "idx": idxh})
    return in_maps, posmaps


def kernel(**inputs):
    global LAST_RESULTS
    x = np.asarray(inputs["input"], dtype=np.float32)
    W = np.asarray(inputs["W"], dtype=np.float32)
    b = np.asarray(inputs["b"], dtype=np.float32)
    part = np.asarray(inputs["partitionings"])
    assert x.shape == (B, D) and W.shape == (P * K, D)

    if "nc" not in _CACHE:
        _CACHE["nc"] = _build_nc()
    nc = _CACHE["nc"]

    in_maps, posmaps = _host_inputs(x, W, b, part)
    trace = bool(int(os.environ.get("BASSK_TRACE", "0")))
    res = bass_utils.run_bass_kernel_spmd(
        nc,
        in_maps,
        core_ids=list(range(NCORES)),
        trace=trace,
        tmpdir=os.environ.get("BASSK_TRACE_DIR") or None,
    )
    LAST_RESULTS = res

    acc = np.zeros((B, C), np.float32)
    for i in range(NCORES):
        sacc = res.results[i]["sacc"].astype(np.float32)
        for h in range(2):
            rec = 1.0 / (sacc[:, 2 * h] + sacc[:, 2 * h + 1])
            raw = res.results[i][f"g{h}"]
            # [part, free] -> [part, g, j, b] -> [b, pos = 8*(g*128+part)+j]
            arr = raw.reshape(128, NOCT // 128, 8, B)
            sv = arr.transpose(3, 1, 0, 2).reshape(B, TOT).astype(np.float32)
            acc += sv[:, posmaps[i][h]] * rec[:, None]
    tot = acc.sum(axis=1, keepdims=True)
    return np.log(acc / tot + ESP).astype(np.float32)


# revision 7
# speedup vs baseline: 2.8052x; 1.5225x over previous
"""Trainium2 Bass kernel for nn_CombinatorialClassifier (segment_reduce), v4.

Strategy (8 NeuronCores, tensor-parallel over the num_partitionings axis):
  Core i owns partitionings {2i, 2i+1} (a [2000, 2048] W slice).

  The segment-gather (out[b,c] = sum_p probs_p[b, idx_p(c)]) runs as a
  SWDGE dma_gather over host-sorted classes:

  - Host sorts each partitioning's classes by partition id k and pads
    every k-run to a multiple of 8, so each group of 8 consecutive
    sorted positions shares one k ("pure oct").
  - Device computes logits = x @ W.T + b, then exp(x - max + ln 128)
    WITHOUT the 1/sum normalization (the per-batch softmax sums ship to
    the host, which applies them during reassembly — exact in fp32).
  - TensorE transposes the fp8-quantized exp values into probsT
    [128, 8, 64] (col k at partition k&127, rank k>>7); 8 strided
    copies replicate each col into an oct row table [1024 rows, 512B]
    (row k = col_k fp8 x8) which is DMA'd to DRAM.
  - dma_gather (HBM source, non-transpose, elem 512B) moves one oct
    row per descriptor: dst[i%128, i//128, 64*j+b] = batch b of sorted
    position 8i+j. 4 calls per half (per-call descriptor-ring cap),
    round-robin over 4 SWDGE queues (safe: non-transpose gathers don't
    touch the shared XBAR transpose unit, unlike transpose mode which
    corrupts data when two queues run concurrently).
  - Sorted fp8 partials [128, TOT/128*512] go to DRAM; the host
    un-permutes (one fancy-index gather per partitioning), applies
    1/sum, accumulates all 16 in fp32, normalizes and takes the log.
"""

import os
from contextlib import ExitStack

import numpy as np

import concourse.bacc as bacc
import concourse.mybir as mybir
import concourse.tile as tile
from concourse import bass_utils

B, P, K, C, D = 64, 16, 1000, 50000, 2048
ESP = 1e-20
NCORES = 8
DCH = D // 128           # 16 contraction chunks of 128
NT = 500                 # matmul N-tile (PSUM bank: 500 fp32 <= 512)
WQ = 4                   # W j-chunks per DMA tile

TOT = 55296              # padded sorted positions per partitioning (8*6912)
NOCT = TOT // 8          # 6912 oct rows per partitioning
OCT_CH = (1024,) * 6 + (768,)       # per-call octs (empirical cap 1024)
IDXW = NOCT // 16        # idx columns per partitioning (432)
NQ = 4                   # SWDGE queues
LN_SCALE = 4.852030263919617        # ln(128): exp scale for fp8 range

_F32 = mybir.dt.float32
_F16 = mybir.dt.float16
_F8 = mybir.dt.float8e4
_I16 = mybir.dt.int16

_CACHE = {}
LAST_RESULTS = None


def _build_nc():
    nc = bacc.Bacc(
        "TRN2",
        target_bir_lowering=False,
        debug=False,
        enable_asserts=False,
        num_devices=NCORES,
        num_swdge_queues=NQ,
    )
    xT_d = nc.dram_tensor("xT", [D, B], _F16, kind="ExternalInput")
    # [2, D+1, K] flattened: per half h, rows h*(D+1)+d = W.T row, last row bias
    wtb_d = nc.dram_tensor("wtb", [2 * (D + 1), K], _F16, kind="ExternalInput")
    id64_d = nc.dram_tensor("id64", [B, B], _F16, kind="ExternalInput")
    idx_d = nc.dram_tensor("idx", [128, 2 * IDXW], _I16, kind="ExternalInput")
    tab_d = [
        nc.dram_tensor(f"tab{h}", [1024, 512], _F8, kind="Internal")
        for h in range(2)
    ]
    out_d = [
        nc.dram_tensor(f"g{h}", [128, (NOCT // 128) * 512], _F8,
                       kind="ExternalOutput")
        for h in range(2)
    ]
    sacc_d = nc.dram_tensor("sacc", [B, 4], _F32, kind="ExternalOutput")

    with tile.TileContext(nc) as tc, ExitStack() as ctx:
        const = ctx.enter_context(tc.tile_pool(name="const", bufs=1))
        wpool = ctx.enter_context(tc.tile_pool(name="w", bufs=6))
        spool = ctx.enter_context(tc.tile_pool(name="stats", bufs=1))
        ppool = ctx.enter_context(tc.tile_pool(name="probs", bufs=2))
        tpool = ctx.enter_context(tc.tile_pool(name="pt", bufs=2))
        tabpool = ctx.enter_context(tc.tile_pool(name="tab", bufs=2))
        gpool = ctx.enter_context(tc.tile_pool(name="g", bufs=5))
        psum = ctx.enter_context(tc.tile_pool(name="psum", bufs=1, space="PSUM"))
        psum2 = ctx.enter_context(tc.tile_pool(name="psum2", bufs=2, space="PSUM"))

        xt = const.tile([128, DCH, B], _F16)
        nc.sync.dma_start(xt[:], xT_d.ap().rearrange("(c p) b -> p c b", p=128))
        ones = const.tile([1, B], _F16)
        nc.vector.memset(ones[:], 1.0)
        bias = const.tile([1, 2, K], _F16)
        nc.sync.dma_start(bias[:, 0, :], wtb_d[D : D + 1, :])
        nc.sync.dma_start(bias[:, 1, :], wtb_d[2 * D + 1 : 2 * D + 2, :])
        id64 = const.tile([B, B], _F16)
        nc.sync.dma_start(id64[:], id64_d.ap())
        idx_sb = const.tile([128, 2 * IDXW], _I16)
        nc.sync.dma_start(idx_sb[:], idx_d.ap())

        mx = spool.tile([B, 4], _F32)
        neg = spool.tile([B, 2], _F32)
        sacc = spool.tile([B, 4], _F32)

        ps = [
            psum.tile([B, NT], _F32, tag=f"ps{n}", name=f"ps{n}")
            for n in range(4)
        ]

        def half(h):
            # ---- logits half h: x @ Wshard[:, hK:(h+1)K].T (+ b) ----
            with nc.named_scope(f"mm{h}"):
                for jq in range(DCH // WQ):
                    wt = wpool.tile([128, WQ, K], _F16, tag="wt", name="wt")
                    eng = nc.sync if jq % 2 == 0 else nc.scalar
                    r0 = h * (D + 1) + 128 * WQ * jq
                    eng.dma_start(
                        wt[:],
                        wtb_d[r0 : r0 + 128 * WQ, :].rearrange(
                            "(c p) k -> p c k", p=128
                        ),
                    )
                    for j in range(WQ):
                        for n in range(2):
                            nc.tensor.matmul(
                                ps[2 * h + n][:],
                                xt[:, WQ * jq + j, :],
                                wt[:, j, NT * n : NT * (n + 1)],
                                start=(jq == 0 and j == 0),
                                stop=False,
                            )
                for n in range(2):
                    nc.tensor.matmul(
                        ps[2 * h + n][:],
                        ones[:],
                        bias[:, h, NT * n : NT * (n + 1)],
                        start=False,
                        stop=True,
                    )

            # ---- scaled exp half h -> probs fp16 [64, K] (unnormalized) ----
            probs = ppool.tile([B, K], _F16, tag="probs", name="probs")
            with nc.named_scope(f"sm{h}"):
                for n in range(2):
                    nc.vector.reduce_max(
                        mx[:, 2 * h + n : 2 * h + n + 1],
                        ps[2 * h + n][:],
                        axis=mybir.AxisListType.X,
                    )
                nc.vector.tensor_tensor(
                    neg[:, h : h + 1],
                    mx[:, 2 * h : 2 * h + 1],
                    mx[:, 2 * h + 1 : 2 * h + 2],
                    op=mybir.AluOpType.max,
                )
                # neg = ln(128) - max  (exp scaled into fp8 range)
                nc.vector.tensor_scalar(
                    neg[:, h : h + 1],
                    neg[:, h : h + 1],
                    -1.0,
                    LN_SCALE,
                    op0=mybir.AluOpType.mult,
                    op1=mybir.AluOpType.add,
                )
                for n in range(2):
                    nc.scalar.activation(
                        probs[:, NT * n : NT * (n + 1)],
                        ps[2 * h + n][:],
                        mybir.ActivationFunctionType.Exp,
                        bias=neg[:, h : h + 1],
                        accum_out=sacc[:, 2 * h + n : 2 * h + n + 1],
                    )

            # ---- transpose: probsT[p, r, b] = probs[b, 128r + p] (fp8) ----
            probsT = tpool.tile([128, 8, B], _F8, tag="pT", name="pT")
            with nc.named_scope(f"tr{h}"):
                for r in range(8):
                    w = min(128, K - 128 * r)
                    pt = psum2.tile([128, B], _F32, tag="pt", name="pt")
                    nc.tensor.matmul(
                        pt[0:w, :],
                        probs[:, 128 * r : 128 * r + w],
                        id64[:],
                        start=True,
                        stop=True,
                    )
                    nc.scalar.copy(probsT[0:w, r, :], pt[0:w, :])

            # ---- oct table: tab[p, kc, j, b] = probsT[p, kc, b], to DRAM ----
            tab = tabpool.tile([128, 8, 8, B], _F8, tag="tab", name="tab")
            with nc.named_scope(f"tab{h}"):
                for j in range(8):
                    eng = nc.scalar if j % 2 == 0 else nc.vector
                    if eng is nc.scalar:
                        eng.copy(tab[:, :, j, :], probsT[:, :, :])
                    else:
                        eng.tensor_copy(tab[:, :, j, :], probsT[:, :, :])
                nc.sync.dma_start(
                    tab_d[h].ap().rearrange("(c p) e -> p c e", p=128),
                    tab[:],
                )

            # ---- gathers: 4 calls, round-robin queues ----
            with nc.named_scope(f"gather{h}"):
                c0 = 0
                for ci, n in enumerate(OCT_CH):
                    dst = gpool.tile(
                        [128, n // 128, 512], _F8, tag=f"dst{n}", name="dst"
                    )
                    nc.gpsimd.dma_gather(
                        dst[:],
                        tab_d[h].ap(),
                        idx_sb[:, h * IDXW + c0 // 16 : h * IDXW + (c0 + n) // 16],
                        n,
                        n,
                        512,
                        queue_num=(h * len(OCT_CH) + ci) % NQ,
                    )
                    eng = nc.sync if ci % 2 == 0 else nc.scalar
                    eng.dma_start(
                        out_d[h][:, (c0 // 128) * 512 : ((c0 + n) // 128) * 512],
                        dst[:],
                    )
                    c0 += n

        half(0)
        half(1)
        nc.sync.dma_start(sacc_d.ap(), sacc[:])

    nc.compile()
    return nc


def _oct_prep(kval):
    """Pure-oct sorted layout for one partitioning.

    kval: [C] ints in [0, K). Pads every k-run to a multiple of 8 and the
    total to TOT. Returns (rows int16 [NOCT], posmap int64 [C]).
    """
    counts = np.bincount(kval, minlength=K)
    pad = (8 - counts % 8) % 8
    total = int(counts.sum() + pad.sum())
    assert total <= TOT, f"pathological partition map: {total} > {TOT}"
    order = np.argsort(kval, kind="stable")
    n_pad = counts + pad
    n_pad[K - 1] += TOT - total
    starts = np.concatenate(([0], np.cumsum(n_pad)[:-1]))
    src_starts = np.concatenate(([0], np.cumsum(counts)[:-1]))
    within = np.arange(C) - np.repeat(src_starts, counts)
    pos = np.repeat(starts, counts) + within          # position of order[j]
    rows = np.repeat(np.arange(K), n_pad)[::8]        # k of each oct
    posmap = np.empty(C, np.int64)
    posmap[order] = pos
    return rows.astype(np.int16), posmap


def _host_inputs(x, W, b, part):
    xT = np.ascontiguousarray(x.T.astype(np.float16))
    id64 = np.eye(B, dtype=np.float16)
    part = np.asarray(part).astype(np.int64, copy=False)

    in_maps, posmaps = [], []
    for i in range(NCORES):
        r0 = 2 * K * i
        wtb = np.empty((2 * (D + 1), K), np.float16)
        for h in range(2):
            rows = slice(h * (D + 1), h * (D + 1) + D)
            wtb[rows] = W[r0 + h * K : r0 + (h + 1) * K].T
            wtb[h * (D + 1) + D] = b[r0 + h * K : r0 + (h + 1) * K]

        idxh = np.zeros((128, 2 * IDXW), np.int16)
        pm = []
        for h in range(2):
            kval = (part[2 * i + h] - (2 * i + h) * K).astype(np.int64)
            rows_o, posmap = _oct_prep(kval)
            blk = rows_o.reshape(IDXW, 16).T
            for q in range(8):
                idxh[16 * q : 16 * q + 16, h * IDXW : (h + 1) * IDXW] = blk
            pm.append(posmap)
        posmaps.append(pm)
        in_maps.append({"xT": xT, "wtb": wtb, "id64": id64, "idx": idxh})
    return in_maps, posmaps


def kernel(**inputs):
    global LAST_RESULTS
    x = np.asarray(inputs["input"], dtype=np.float32)
    W = np.asarray(inputs["W"], dtype=np.float32)
    b = np.asarray(inputs["b"], dtype=np.float32)
    part = np.asarray(inputs["partitionings"])
    assert x.shape == (B, D) and W.shape == (P * K, D)

    if "nc" not in _CACHE:
        _CACHE["nc"] = _build_nc()
    nc = _CACHE["nc"]

    in_maps, posmaps = _host_inputs(x, W, b, part)
    trace = bool(int(os.environ.get("BASSK_TRACE", "0")))
    res = bass_utils.run_bass_kernel_spmd(
        nc,
        in_maps,
        core_ids=list(range(NCORES)),
        trace=trace,
        tmpdir=os.environ.get("BASSK_TRACE_DIR") or None,
    )
    LAST_RESULTS = res

    acc = np.zeros((B, C), np.float32)
    for i in range(NCORES):
        sacc = res.results[i]["sacc"].astype(np.float32)
        for h in range(2):
            rec = 1.0 / (sacc[:, 2 * h] + sacc[:, 2 * h + 1])
            raw = res.results[i][f"g{h}"]
            # [part, free] -> [part, g, j, b] -> [b, pos = 8*(g*128+part)+j]
            arr = raw.reshape(128, NOCT // 128, 8, B)
            sv = arr.transpose(3, 1, 0, 2).reshape(B, TOT).astype(np.float32)
            acc += sv[:, posmaps[i][h]] * rec[:, None]
    tot = acc.sum(axis=1, keepdims=True)
    return np.log(acc / tot + ESP).astype(np.float32)
